# revision 2
# baseline (speedup 1.0000x reference)
"""Gated MLP (SwiGLU) on 8 TRN2 NeuronCores, tensor-parallel over the
intermediate dimension.

Math (per reference): g = x @ Wg.T ; u = x @ Wu.T ; a = silu(g)*u ;
d = a @ Wd.T, with x:[2,2048,4096] f32, Wg/Wu:[14336,4096], Wd:[4096,14336].

Sharding: core c owns intermediate slice I_c = c*1792:(c+1)*1792. Each core
computes gT/uT/aT for its slice against all 4096 tokens, then a partial
dT[c] = WdT[I_c,:].T-contraction. Host sums the 8 partials (the tp_reduce)
and transposes back.

On-chip layout (everything transposed so contractions land on partitions):
  xT  [H=4096, T=4096] bf16            (rhs for gate/up)
  wg/wu [14, 128, 4096] bf16 pre-tiled (lhsT [k128, i128] stationary;
                                        wg[i, p, k*128+m] = Wg.T[k*128+p, i*128+m])
  wd  [32, 128, 1792] bf16 pre-tiled   (lhsT [i128, h128] stationary)
  out [H, T] f32 partial               (dT; host reduces + transposes)

The kernel is PE-bound at the bf16 roofline (~2.29ms of pure streaming), so
the schedule focuses on eliminating PE idle time:
  - warmup matmuls on scratch SBUF at t=0 keep the PE busy through the
    initial DMA wait and get the HAM clock gate to 8/8 before real work
  - q0's activation tiles are striped across the sync and gpsimd DMA
    queues (ahead of any wd prefetch) and the first gate/up weight tiles
    are k-chunked so the first real matmul can start as soon as ~300KB
    has landed; the ramp is HBM-bound either way
  - the down projection runs n-outer/i-inner against two single-bank PSUM
    tiles so each 512-column chunk is copied out and DMA'd (on the HWDGE
    sync/scalar queues) while the next chunk accumulates — the end-of-
    kernel drain is one 512-col copy + DMA instead of a full h-tile
"""

import sys

if "/opt/trn_rl_repo" not in sys.path:
    sys.path.insert(0, "/opt/trn_rl_repo")

import numpy as np
import ml_dtypes

H = 4096          # hidden
I_FULL = 14336    # intermediate
T = 4096          # tokens (2*2048)
NCORES = 8
ISH = I_FULL // NCORES   # 1792 per-core intermediate slice
P = 128
QT = 1024         # tokens per outer block
NQ = T // QT      # 4
KT = H // P       # 32 contraction tiles for gate/up
IT = ISH // P     # 14 contraction tiles for down
HT = H // P       # 32 output-row tiles for down
NF = 512          # matmul moving free-dim (one PSUM bank of f32)
NWARM = 12        # warmup matmuls (~4-5us of PE busy from t=0)

_BUILT = {}


def _build():
    if "nc" in _BUILT:
        return _BUILT["nc"]
    from concourse import bacc
    import concourse.mybir as mybir
    import concourse.tile as tile
    from contextlib import ExitStack

    bf = mybir.dt.bfloat16
    f32 = mybir.dt.float32
    nc = bacc.Bacc(
        "TRN2",
        target_bir_lowering=False,
        debug=False,
        enable_asserts=False,
        num_devices=NCORES,
    )

    xT = nc.dram_tensor("xT", [H, T], bf, kind="ExternalInput").ap()
    wg = nc.dram_tensor("wg", [IT, P, KT * P], bf, kind="ExternalInput").ap()
    wu = nc.dram_tensor("wu", [IT, P, KT * P], bf, kind="ExternalInput").ap()
    wd = nc.dram_tensor("wd", [HT, P, IT * P], bf, kind="ExternalInput").ap()
    out = nc.dram_tensor("out", [H, T], f32, kind="ExternalOutput").ap()

    # [p, k, t] view: per-partition rows stay contiguous in t
    x_r = xT.rearrange("(k p) t -> p k t", p=P)     # [128, 32, 4096]

    with tile.TileContext(nc) as tc, ExitStack() as ctx:
        warm_pool = ctx.enter_context(tc.tile_pool(name="warm", bufs=1))
        xt_pool = ctx.enter_context(tc.tile_pool(name="xt", bufs=KT + 6))
        wg_pool = ctx.enter_context(tc.tile_pool(name="wg", bufs=2))
        wu_pool = ctx.enter_context(tc.tile_pool(name="wu", bufs=2))
        wd_pool = ctx.enter_context(tc.tile_pool(name="wd", bufs=3))
        at_pool = ctx.enter_context(tc.tile_pool(name="at", bufs=IT + 1))
        tmp_pool = ctx.enter_context(tc.tile_pool(name="tmp", bufs=2))
        dst_pool = ctx.enter_context(tc.tile_pool(name="dst", bufs=4))
        pw_pool = ctx.enter_context(tc.tile_pool(name="pw", bufs=1, space="PSUM"))
        pg_pool = ctx.enter_context(tc.tile_pool(name="pg", bufs=1, space="PSUM"))
        pu_pool = ctx.enter_context(tc.tile_pool(name="pu", bufs=1, space="PSUM"))
        pd_pool = ctx.enter_context(tc.tile_pool(name="pd", bufs=2, space="PSUM"))

        # ---- PE warmup: matmuls on scratch data, no DMA dependency ----
        wsrc = warm_pool.tile([P, P + NF], bf)
        nc.vector.memset(wsrc[:], 0)
        pw = pw_pool.tile([P, NF], f32)
        for _ in range(NWARM):
            nc.tensor.matmul(
                pw[:], wsrc[:, 0:P], wsrc[:, P : P + NF], start=True, stop=True
            )

        def load_w(pool, src, i):
            t = pool.tile([P, KT, P], bf)
            # src[i] is [128, 4096] contiguous per partition
            nc.scalar.dma_start(out=t[:], in_=src[i].rearrange("p (k m) -> p k m", m=P))
            return t

        for q in range(NQ):
            t0 = q * QT

            # first gate/up weights go out before the xT block so the PE can
            # start as soon as xt[0] lands; at kernel start, stage them in
            # interleaved 8-k chunks so the first chunks of both arrive early
            if q == 0:
                wg_t = wg_pool.tile([P, KT, P], bf)
                wu_t = wu_pool.tile([P, KT, P], bf)
                wgv = wg[0].rearrange("p (k m) -> p k m", m=P)
                wuv = wu[0].rearrange("p (k m) -> p k m", m=P)
                ck = 8
                for c in range(KT // ck):
                    nc.scalar.dma_start(
                        out=wg_t[:, c * ck : (c + 1) * ck, :],
                        in_=wgv[:, c * ck : (c + 1) * ck, :],
                    )
                    nc.scalar.dma_start(
                        out=wu_t[:, c * ck : (c + 1) * ck, :],
                        in_=wuv[:, c * ck : (c + 1) * ck, :],
                    )
            else:
                wg_t = load_w(wg_pool, wg, 0)
                wu_t = load_w(wu_pool, wu, 0)

            # stage this block's activations: 32 k-tiles of [128, QT].
            # At kernel start the x stream is the ramp critical path — stripe
            # it across the sync HWDGE queue and the gpsimd SWDGE queue
            # (ahead of any wd prefetch in gpsimd program order).
            xts = []
            for k in range(KT):
                xt_t = xt_pool.tile([P, QT], bf)
                eng = nc.gpsimd if (q == 0 and k % 2 == 1) else nc.sync
                eng.dma_start(out=xt_t[:], in_=x_r[:, k, t0 : t0 + QT])
                xts.append(xt_t)

            # ---- gate/up + silu*mul, producing aT[i] tiles ----
            ats = []
            for i in range(IT):
                if i > 0:
                    wg_t = load_w(wg_pool, wg, i)
                    wu_t = load_w(wu_pool, wu, i)
                pg = pg_pool.tile([P, QT], f32)
                if q == 0 and i == 0:
                    # kernel start: xt tiles arrive at HBM rate — interleave
                    # g and u per k so PE consumption stays behind arrival
                    pu = pu_pool.tile([P, QT], f32)
                    for k in range(KT):
                        for w_t, ps in ((wg_t, pg), (wu_t, pu)):
                            for n in range(QT // NF):
                                nc.tensor.matmul(
                                    ps[:, n * NF : (n + 1) * NF],
                                    w_t[:, k, :],
                                    xts[k][:, n * NF : (n + 1) * NF],
                                    start=(k == 0),
                                    stop=(k == KT - 1),
                                )
                    tmp = tmp_pool.tile([P, QT], bf)
                    nc.scalar.activation(
                        tmp[:], pg[:], mybir.ActivationFunctionType.Silu
                    )
                else:
                    for k in range(KT):
                        for n in range(QT // NF):
                            nc.tensor.matmul(
                                pg[:, n * NF : (n + 1) * NF],
                                wg_t[:, k, :],
                                xts[k][:, n * NF : (n + 1) * NF],
                                start=(k == 0),
                                stop=(k == KT - 1),
                            )
                    # silu(g) on ScalarE while the u matmuls run
                    tmp = tmp_pool.tile([P, QT], bf)
                    nc.scalar.activation(
                        tmp[:], pg[:], mybir.ActivationFunctionType.Silu
                    )
                    pu = pu_pool.tile([P, QT], f32)
                    for k in range(KT):
                        for n in range(QT // NF):
                            nc.tensor.matmul(
                                pu[:, n * NF : (n + 1) * NF],
                                wu_t[:, k, :],
                                xts[k][:, n * NF : (n + 1) * NF],
                                start=(k == 0),
                                stop=(k == KT - 1),
                            )
                at = at_pool.tile([P, QT], bf)
                nc.vector.tensor_tensor(
                    at[:], tmp[:], pu[:], mybir.AluOpType.mult
                )
                ats.append(at)

            # ---- down projection: dT[h, t] partial ----
            # n-outer: each 512-col chunk accumulates into its own PSUM bank,
            # is copied to SBUF while the next chunk's matmuls run, and DMAs
            # out on the HWDGE queues so the tail drain is one chunk deep.
            for h in range(HT):
                h0 = h * P
                wd_t = wd_pool.tile([P, IT, P], bf)
                nc.gpsimd.dma_start(
                    out=wd_t[:], in_=wd[h].rearrange("p (i m) -> p i m", m=P)
                )
                for n in range(QT // NF):
                    ns = slice(n * NF, (n + 1) * NF)
                    pd = pd_pool.tile([P, NF], f32)
                    for i in range(IT):
                        nc.tensor.matmul(
                            pd[:],
                            wd_t[:, i, :],
                            ats[i][:, ns],
                            start=(i == 0),
                            stop=(i == IT - 1),
                        )
                    dst = dst_pool.tile([P, NF], f32)
                    nc.vector.tensor_copy(dst[:], pd[:])
                    eng = nc.sync if h % 2 == 0 else nc.scalar
                    eng.dma_start(
                        out=out[h0 : h0 + P, t0 + n * NF : t0 + (n + 1) * NF],
                        in_=dst[:],
                    )

    nc.compile()
    _BUILT["nc"] = nc
    return nc


def _prep_inputs(x, Wg, Wu, Wd):
    bf = ml_dtypes.bfloat16
    xTn = x.reshape(T, H).T.astype(bf, order="C")        # [H, T]
    # single-pass cast + shard + pre-tile:
    #   wg[c][i, p, k*128+m] = Wg.T[k*128+p, c*1792 + i*128+m]
    wg_all = np.ascontiguousarray(
        Wg.reshape(NCORES, IT, P, KT, P).transpose(0, 1, 4, 3, 2), dtype=bf
    ).reshape(NCORES, IT, P, KT * P)
    wu_all = np.ascontiguousarray(
        Wu.reshape(NCORES, IT, P, KT, P).transpose(0, 1, 4, 3, 2), dtype=bf
    ).reshape(NCORES, IT, P, KT * P)
    #   wd[c][h, p, i*128+m] = Wd.T[c*1792 + i*128+p, h*128+m]
    wd_all = np.ascontiguousarray(
        Wd.reshape(HT, P, NCORES, IT, P).transpose(2, 0, 4, 3, 1), dtype=bf
    ).reshape(NCORES, HT, P, IT * P)
    return [
        {"xT": xTn, "wg": wg_all[c], "wu": wu_all[c], "wd": wd_all[c]}
        for c in range(NCORES)
    ]


def _run(in_maps, **kw):
    from concourse.bass_utils import run_bass_kernel_spmd

    nc = _build()
    return run_bass_kernel_spmd(nc, in_maps, core_ids=list(range(NCORES)), **kw)


def _gather(results, batch_shape):
    acc = results[0]["out"].astype(np.float32)
    for r in results[1:]:
        acc += r["out"]
    return np.ascontiguousarray(acc.T).reshape(batch_shape)


def kernel(x, Wg, Wu, Wd):
    x = np.asarray(x)
    in_maps = _prep_inputs(
        np.asarray(x, dtype=np.float32),
        np.asarray(Wg, dtype=np.float32),
        np.asarray(Wu, dtype=np.float32),
        np.asarray(Wd, dtype=np.float32),
    )
    res = _run(in_maps)
    return _gather(res.results, x.shape)


# revision 6
# speedup vs baseline: 1.0058x; 1.0058x over previous
"""Gated MLP (SwiGLU) on 8 TRN2 NeuronCores, tensor-parallel over the
intermediate dimension.

Math (per reference): g = x @ Wg.T ; u = x @ Wu.T ; a = silu(g)*u ;
d = a @ Wd.T, with x:[2,2048,4096] f32, Wg/Wu:[14336,4096], Wd:[4096,14336].

Sharding: core c owns intermediate slice I_c = c*1792:(c+1)*1792. Each core
computes gT/uT/aT for its slice against all 4096 tokens, then a partial
dT[c] = WdT[I_c,:].T-contraction. Host sums the 8 partials (the tp_reduce)
and transposes back.

On-chip layout (everything transposed so contractions land on partitions):
  xT  [H=4096, T=4096] bf16            (rhs for gate/up)
  wg/wu [14, 128, 4096] bf16 pre-tiled (lhsT [k128, i128] stationary;
                                        wg[i, p, k*128+m] = Wg.T[k*128+p, i*128+m])
  wd  [32, 128, 1792] bf16 pre-tiled   (lhsT [i128, h128] stationary)
  out [H, T] f32 partial               (dT; host reduces + transposes)

The kernel is PE-bound at the bf16 roofline (~2.29ms of pure streaming), so
the schedule focuses on eliminating PE idle time:
  - warmup matmuls on scratch SBUF at t=0 keep the PE busy through the
    initial DMA wait and get the HAM clock gate to 8/8 before real work
  - q0's activation tiles are striped across the sync and gpsimd DMA
    queues (ahead of any wd prefetch) and the first gate/up weight tiles
    are k-chunked so the first real matmul can start as soon as ~300KB
    has landed; the ramp is HBM-bound either way
  - the down projection runs n-outer/i-inner against two single-bank PSUM
    tiles so each 512-column chunk is copied out and DMA'd (on the HWDGE
    sync/scalar queues) while the next chunk accumulates — the end-of-
    kernel drain is one 512-col copy + DMA instead of a full h-tile
"""

import sys

if "/opt/trn_rl_repo" not in sys.path:
    sys.path.insert(0, "/opt/trn_rl_repo")

import numpy as np
import ml_dtypes

H = 4096          # hidden
I_FULL = 14336    # intermediate
T = 4096          # tokens (2*2048)
NCORES = 8
ISH = I_FULL // NCORES   # 1792 per-core intermediate slice
P = 128
QT = 1024         # tokens per outer block
NQ = T // QT      # 4
KT = H // P       # 32 contraction tiles for gate/up
IT = ISH // P     # 14 contraction tiles for down
HT = H // P       # 32 output-row tiles for down
NF = 512          # matmul moving free-dim (one PSUM bank of f32)
NWARM = 12        # warmup matmuls (~4-5us of PE busy from t=0)

_BUILT = {}


def _build():
    if "nc" in _BUILT:
        return _BUILT["nc"]
    from concourse import bacc
    import concourse.mybir as mybir
    import concourse.tile as tile
    from contextlib import ExitStack

    bf = mybir.dt.bfloat16
    f32 = mybir.dt.float32
    nc = bacc.Bacc(
        "TRN2",
        target_bir_lowering=False,
        debug=False,
        enable_asserts=False,
        num_devices=NCORES,
    )

    xT = nc.dram_tensor("xT", [H, T], bf, kind="ExternalInput").ap()
    wg = nc.dram_tensor("wg", [IT, P, KT * P], bf, kind="ExternalInput").ap()
    wu = nc.dram_tensor("wu", [IT, P, KT * P], bf, kind="ExternalInput").ap()
    wd = nc.dram_tensor("wd", [HT, P, IT * P], bf, kind="ExternalInput").ap()
    out = nc.dram_tensor("out", [H, T], f32, kind="ExternalOutput").ap()

    # [p, k, t] view: per-partition rows stay contiguous in t
    x_r = xT.rearrange("(k p) t -> p k t", p=P)     # [128, 32, 4096]

    with tile.TileContext(nc) as tc, ExitStack() as ctx:
        xt_pool = ctx.enter_context(tc.tile_pool(name="xt", bufs=KT + 6))
        wg_pool = ctx.enter_context(tc.tile_pool(name="wg", bufs=2))
        wu_pool = ctx.enter_context(tc.tile_pool(name="wu", bufs=2))
        wd_pool = ctx.enter_context(tc.tile_pool(name="wd", bufs=3))
        at_pool = ctx.enter_context(tc.tile_pool(name="at", bufs=IT + 1))
        tmp_pool = ctx.enter_context(tc.tile_pool(name="tmp", bufs=2))
        dst_pool = ctx.enter_context(tc.tile_pool(name="dst", bufs=4))
        pg_pool = ctx.enter_context(tc.tile_pool(name="pg", bufs=1, space="PSUM"))
        pu_pool = ctx.enter_context(tc.tile_pool(name="pu", bufs=1, space="PSUM"))
        pd_pool = ctx.enter_context(tc.tile_pool(name="pd", bufs=4, space="PSUM"))

        def load_w(pool, src, i):
            t = pool.tile([P, KT, P], bf)
            # src[i] is [128, 4096] contiguous per partition
            nc.scalar.dma_start(out=t[:], in_=src[i].rearrange("p (k m) -> p k m", m=P))
            return t

        for q in range(NQ):
            t0 = q * QT

            # first gate/up weights go out before the xT block so the PE can
            # start as soon as xt[0] lands; at kernel start, stage them in
            # interleaved 8-k chunks so the first chunks of both arrive early
            if q == 0:
                wg_t = wg_pool.tile([P, KT, P], bf)
                wu_t = wu_pool.tile([P, KT, P], bf)
                wgv = wg[0].rearrange("p (k m) -> p k m", m=P)
                wuv = wu[0].rearrange("p (k m) -> p k m", m=P)
                ck = 8
                for c in range(KT // ck):
                    nc.scalar.dma_start(
                        out=wg_t[:, c * ck : (c + 1) * ck, :],
                        in_=wgv[:, c * ck : (c + 1) * ck, :],
                    )
                    nc.scalar.dma_start(
                        out=wu_t[:, c * ck : (c + 1) * ck, :],
                        in_=wuv[:, c * ck : (c + 1) * ck, :],
                    )
            else:
                wg_t = load_w(wg_pool, wg, 0)
                wu_t = load_w(wu_pool, wu, 0)

            # stage this block's activations: 32 k-tiles of [128, QT]
            xts = []
            for k in range(KT):
                xt_t = xt_pool.tile([P, QT], bf)
                nc.sync.dma_start(out=xt_t[:], in_=x_r[:, k, t0 : t0 + QT])
                xts.append(xt_t)

            # ---- gate/up + silu*mul, producing aT[i] tiles ----
            ats = []
            i_start = 0
            if q == 0:
                # Kernel-start ramp: the PE is DMA-bound while q0's weights
                # and xt tiles stream in, so interleave the i=0 AND i=1
                # gate+up accumulations over the k loop — 8 matmuls (~1.7us)
                # of PE work per arriving xt tile instead of 4.  i=0 uses the
                # pg/pu banks; i=1 borrows four single-bank tiles from the
                # down-projection pool (idle during the gate phase).
                wg_t1 = wg_pool.tile([P, KT, P], bf)
                wu_t1 = wu_pool.tile([P, KT, P], bf)
                wgv1 = wg[1].rearrange("p (k m) -> p k m", m=P)
                wuv1 = wu[1].rearrange("p (k m) -> p k m", m=P)
                ck = 8
                for c in range(KT // ck):
                    cs = slice(c * ck, (c + 1) * ck)
                    nc.scalar.dma_start(out=wg_t1[:, cs, :], in_=wgv1[:, cs, :])
                    nc.scalar.dma_start(out=wu_t1[:, cs, :], in_=wuv1[:, cs, :])

                pg0 = pg_pool.tile([P, QT], f32)
                pu0 = pu_pool.tile([P, QT], f32)
                pg1n = [pd_pool.tile([P, NF], f32) for _ in range(2)]
                pu1n = [pd_pool.tile([P, NF], f32) for _ in range(2)]
                for k in range(KT):
                    st, sp = (k == 0), (k == KT - 1)
                    for n in range(QT // NF):
                        ns = slice(n * NF, (n + 1) * NF)
                        nc.tensor.matmul(
                            pg0[:, ns], wg_t[:, k, :], xts[k][:, ns],
                            start=st, stop=sp,
                        )
                    for n in range(QT // NF):
                        ns = slice(n * NF, (n + 1) * NF)
                        nc.tensor.matmul(
                            pu0[:, ns], wu_t[:, k, :], xts[k][:, ns],
                            start=st, stop=sp,
                        )
                    for n in range(QT // NF):
                        ns = slice(n * NF, (n + 1) * NF)
                        nc.tensor.matmul(
                            pg1n[n][:], wg_t1[:, k, :], xts[k][:, ns],
                            start=st, stop=sp,
                        )
                    for n in range(QT // NF):
                        ns = slice(n * NF, (n + 1) * NF)
                        nc.tensor.matmul(
                            pu1n[n][:], wu_t1[:, k, :], xts[k][:, ns],
                            start=st, stop=sp,
                        )
                tmp0 = tmp_pool.tile([P, QT], bf)
                nc.scalar.activation(
                    tmp0[:], pg0[:], mybir.ActivationFunctionType.Silu
                )
                at0 = at_pool.tile([P, QT], bf)
                nc.vector.tensor_tensor(
                    at0[:], tmp0[:], pu0[:], mybir.AluOpType.mult
                )
                tmp1 = tmp_pool.tile([P, QT], bf)
                at1 = at_pool.tile([P, QT], bf)
                for n in range(QT // NF):
                    ns = slice(n * NF, (n + 1) * NF)
                    nc.scalar.activation(
                        tmp1[:, ns], pg1n[n][:], mybir.ActivationFunctionType.Silu
                    )
                    nc.vector.tensor_tensor(
                        at1[:, ns], tmp1[:, ns], pu1n[n][:], mybir.AluOpType.mult
                    )
                ats += [at0, at1]
                i_start = 2

            for i in range(i_start, IT):
                if i > 0:
                    wg_t = load_w(wg_pool, wg, i)
                    wu_t = load_w(wu_pool, wu, i)
                pg = pg_pool.tile([P, QT], f32)
                if True:
                    for k in range(KT):
                        for n in range(QT // NF):
                            nc.tensor.matmul(
                                pg[:, n * NF : (n + 1) * NF],
                                wg_t[:, k, :],
                                xts[k][:, n * NF : (n + 1) * NF],
                                start=(k == 0),
                                stop=(k == KT - 1),
                            )
                    # silu(g) on ScalarE while the u matmuls run
                    tmp = tmp_pool.tile([P, QT], bf)
                    nc.scalar.activation(
                        tmp[:], pg[:], mybir.ActivationFunctionType.Silu
                    )
                    pu = pu_pool.tile([P, QT], f32)
                    for k in range(KT):
                        for n in range(QT // NF):
                            nc.tensor.matmul(
                                pu[:, n * NF : (n + 1) * NF],
                                wu_t[:, k, :],
                                xts[k][:, n * NF : (n + 1) * NF],
                                start=(k == 0),
                                stop=(k == KT - 1),
                            )
                at = at_pool.tile([P, QT], bf)
                nc.vector.tensor_tensor(
                    at[:], tmp[:], pu[:], mybir.AluOpType.mult
                )
                ats.append(at)

            # ---- down projection: dT[h, t] partial ----
            # n-outer: each 512-col chunk accumulates into its own PSUM bank,
            # is copied to SBUF while the next chunk's matmuls run, and DMAs
            # out on the HWDGE queues so the tail drain is one chunk deep.
            for h in range(HT):
                h0 = h * P
                wd_t = wd_pool.tile([P, IT, P], bf)
                nc.gpsimd.dma_start(
                    out=wd_t[:], in_=wd[h].rearrange("p (i m) -> p i m", m=P)
                )
                for n in range(QT // NF):
                    ns = slice(n * NF, (n + 1) * NF)
                    pd = pd_pool.tile([P, NF], f32)
                    for i in range(IT):
                        nc.tensor.matmul(
                            pd[:],
                            wd_t[:, i, :],
                            ats[i][:, ns],
                            start=(i == 0),
                            stop=(i == IT - 1),
                        )
                    dst = dst_pool.tile([P, NF], f32)
                    nc.vector.tensor_copy(dst[:], pd[:])
                    # outputs ride gpsimd (keeps the HWDGE queues clear for
                    # the xt/weight streams); the last h-tile's chunks go on
                    # sync/scalar so the kernel tail skips SWDGE latency
                    if q == NQ - 1 and h == HT - 1:
                        eng = nc.sync if n == 0 else nc.scalar
                    else:
                        eng = nc.gpsimd
                    eng.dma_start(
                        out=out[h0 : h0 + P, t0 + n * NF : t0 + (n + 1) * NF],
                        in_=dst[:],
                    )

    nc.compile()
    _BUILT["nc"] = nc
    return nc


def _prep_inputs(x, Wg, Wu, Wd):
    bf = ml_dtypes.bfloat16
    xTn = x.reshape(T, H).T.astype(bf, order="C")        # [H, T]
    # single-pass cast + shard + pre-tile:
    #   wg[c][i, p, k*128+m] = Wg.T[k*128+p, c*1792 + i*128+m]
    wg_all = np.ascontiguousarray(
        Wg.reshape(NCORES, IT, P, KT, P).transpose(0, 1, 4, 3, 2), dtype=bf
    ).reshape(NCORES, IT, P, KT * P)
    wu_all = np.ascontiguousarray(
        Wu.reshape(NCORES, IT, P, KT, P).transpose(0, 1, 4, 3, 2), dtype=bf
    ).reshape(NCORES, IT, P, KT * P)
    #   wd[c][h, p, i*128+m] = Wd.T[c*1792 + i*128+p, h*128+m]
    wd_all = np.ascontiguousarray(
        Wd.reshape(HT, P, NCORES, IT, P).transpose(2, 0, 4, 3, 1), dtype=bf
    ).reshape(NCORES, HT, P, IT * P)
    return [
        {"xT": xTn, "wg": wg_all[c], "wu": wu_all[c], "wd": wd_all[c]}
        for c in range(NCORES)
    ]


def _run(in_maps, **kw):
    from concourse.bass_utils import run_bass_kernel_spmd

    nc = _build()
    return run_bass_kernel_spmd(nc, in_maps, core_ids=list(range(NCORES)), **kw)


def _gather(results, batch_shape):
    acc = results[0]["out"].astype(np.float32)
    for r in results[1:]:
        acc += r["out"]
    return np.ascontiguousarray(acc.T).reshape(batch_shape)


def kernel(x, Wg, Wu, Wd):
    x = np.asarray(x)
    in_maps = _prep_inputs(
        np.asarray(x, dtype=np.float32),
        np.asarray(Wg, dtype=np.float32),
        np.asarray(Wu, dtype=np.float32),
        np.asarray(Wd, dtype=np.float32),
    )
    res = _run(in_maps)
    return _gather(res.results, x.shape)
